# revision 38
# baseline (speedup 1.0000x reference)
"""Trainium2 Bass kernel for the K1CompleteSystem dense-MLP problem.

Data-parallel over tokens: 4096 tokens split as 512/core across 8 cores.
All (tiny) agent weights replicated; each core computes its token slice of
the full-vocab logits. Agent affine params are folded into effective
weights/biases on the host (exact algebra, no approximation):

  hid_a  = gelu(xhat @ (g1_a*W1_a) + (b1ln_a@W1_a + bfc1_a))
  out_a  = ln(hid_a) @ (g2_a*W2_a) + (b2ln_a@W2_a + bfc2_a) + flat
  hidden = 2*flat + mean_a(ln0(hid_a) @ W2e_a) + b2avg
  logits = hidden @ Wout + bout

b2avg is folded into the device-side residual (per-partition bias on the
feature-major hidden), so the only remaining vocab bias is bout itself,
which is added on the host afterwards iff nonzero (it is zero in the
reference setup).

The target_regime is memory: the 512x32000 fp32 logits write dominates.
The device computes/stores logits in float16 (rel err ~5e-4 << the 2e-2
gate) halving the dominant HBM traffic, and the host upcasts. Wout is
also fp16 and fully resident in SBUF (64KB/partition), prefetched during
the MLP phases, so the logits loop has no input-DMA dependency and the
PE stays warm. All MLP matmuls run fp16 (1 cycle/row).
"""

import os
import sys
from contextlib import ExitStack

import numpy as np

for _p in ("/opt/trn_rl_repo",):
    if _p not in sys.path and os.path.isdir(_p):
        sys.path.insert(0, _p)

try:
    import concourse.bass as bass
    import concourse.tile as tile
    from concourse import mybir
    from concourse.bass_utils import run_bass_kernel_spmd
    from concourse.masks import make_identity
    _HAVE_BASS = True
except Exception:
    _HAVE_BASS = False

A = 21
D = 128
H = 256
V = 32000
EPS = 1e-5
NCORES = 8
NTOK = 512          # tokens per core
NT = NTOK // 128    # token tiles per core
if _HAVE_BASS:
    F32 = mybir.dt.float32
    F16 = mybir.dt.float16

# logits: per token tile, 4 DMA stages; 512-wide matmul sub-chunks
# (one PSUM bank each; the tail stage has a 256-wide remainder)
STAGES = [(0, 8192, [512] * 16), (8192, 8192, [512] * 16),
          (16384, 8192, [512] * 16), (24576, 7424, [512] * 14 + [256])]
assert sum(w for _, w, _ in STAGES) == V

_CACHE: dict = {}


def _split_multi_waits(nc) -> int:
    """This container's walrus allows at most ONE sync-wait command per
    instruction ("Too many sync wait commands"). Tile freely fuses several
    waits onto one instruction; hoist all but the last onto single-wait
    NoOps placed immediately before it on the same (in-order) engine
    queue — semantically identical."""
    n_split = 0
    for func in nc.m.functions:
        for block in func.blocks:
            out = []
            for inst in block.instructions:
                si = inst.sync_info
                if si is not None and si.on_wait and len(si.on_wait) > 1:
                    waits = list(si.on_wait)
                    for w in waits[:-1]:
                        out.append(
                            mybir.InstNoOp(
                                name=nc.get_next_instruction_name(),
                                ins=[],
                                outs=[],
                                engine=inst.engine,
                                sync_info=mybir.SyncInfo(on_wait=[w], on_update=[]),
                                bass_nofuse=True,
                            )
                        )
                        n_split += 1
                    inst.sync_info = mybir.SyncInfo(
                        on_wait=[waits[-1]], on_update=list(si.on_update)
                    )
                out.append(inst)
            block.instructions = out
    return n_split


def _build_nc() -> bass.Bass:
    nc = bass.Bass("TRN2")

    idx_d = nc.declare_dram_parameter("idx", [128, NT], mybir.dt.int32, isOutput=False)
    ones_d = nc.declare_dram_parameter("ones16", [128, 128], F16, isOutput=False)
    b2e_d = nc.declare_dram_parameter("b2eps", [128, 2], F32, isOutput=False)
    emb_d = nc.declare_dram_parameter("emb", [V, D], F32, isOutput=False)
    w1e_d = nc.declare_dram_parameter("w1e", [D, A, H], F16, isOutput=False)
    b1e_d = nc.declare_dram_parameter("b1e", [128, A, 2], F32, isOutput=False)
    w2e_d = nc.declare_dram_parameter("w2e", [128, A, 2, D], F16, isOutput=False)
    wout_d = nc.declare_dram_parameter("wout", [D, V], F16, isOutput=False)
    out_d = nc.declare_dram_parameter("logits", [NTOK, V], F16, isOutput=True)

    sub = mybir.AluOpType.subtract
    mult = mybir.AluOpType.mult
    add = mybir.AluOpType.add
    Ln = mybir.ActivationFunctionType.Ln
    Exp = mybir.ActivationFunctionType.Exp
    Gelu = mybir.ActivationFunctionType.Gelu
    Ident = mybir.ActivationFunctionType.Identity

    with tile.TileContext(nc) as tc, ExitStack() as ctx:
        const = ctx.enter_context(tc.tile_pool(name="const", bufs=1))
        big = ctx.enter_context(tc.tile_pool(name="big", bufs=1))
        work = ctx.enter_context(tc.tile_pool(name="work", bufs=2))

        # ---- whole fp16 Wout resident in SBUF; prefetch starts immediately
        # and overlaps all of phases A-D (8 DMAs of 1MB each).
        wout_sb = big.tile([128, V], F16)
        for k in range(8):
            w0 = k * (V // 8)
            nc.sync.dma_start(
                out=wout_sb[:, w0 : w0 + V // 8], in_=wout_d[:, w0 : w0 + V // 8]
            )

        # ---- resident constants / weights ----
        idx_sb = const.tile([128, NT], mybir.dt.int32)
        nc.sync.dma_start(out=idx_sb[:], in_=idx_d[:])
        onesH = const.tile([128, 128], F16)
        nc.sync.dma_start(out=onesH[:], in_=ones_d[:])
        b2e_sb = const.tile([128, 2], F32)
        nc.sync.dma_start(out=b2e_sb[:], in_=b2e_d[:])
        b2avg_col = b2e_sb[:, 0:1]
        eps_col = b2e_sb[:, 1:2]
        w1e_sb = const.tile([D, A, H], F16)
        nc.sync.dma_start(out=w1e_sb[:], in_=w1e_d[:])
        b1e_sb = const.tile([128, A, 2], F32)
        nc.sync.dma_start(out=b1e_sb[:], in_=b1e_d[:])
        w2e_sb = const.tile([128, A, 2, D], F16)
        nc.sync.dma_start(out=w2e_sb[:], in_=w2e_d[:])

        ident = const.tile([128, 128], F32)
        make_identity(nc, ident[:])

        flat_sb = const.tile([128, NT, D], F32)
        xhat_sb = const.tile([128, NT, D], F32)
        xhatT = const.tile([D, NTOK], F16)
        flatT2 = const.tile([D, NTOK], F32)
        hiddenT = const.tile([D, NTOK], F16)

        # ---- phase A: embedding gather + LN1 (token-major) + transposes ----
        with tc.tile_pool(name="psA", bufs=2, space="PSUM") as psA:
            for j in range(NT):
                nc.gpsimd.indirect_dma_start(
                    out=flat_sb[:, j, :],
                    out_offset=None,
                    in_=emb_d[:],
                    in_offset=bass.IndirectOffsetOnAxis(ap=idx_sb[:, j : j + 1], axis=0),
                )
            # rstd = 1/sqrt(var+eps) = exp(-0.5*ln(var+eps)) — two ACT table
            # ops (scalar Rsqrt/Reciprocal are API-blocked, the custom-DVE
            # reciprocal fails this walrus's codegen). Batched over all NT
            # token tiles so the Ln/Exp tables load once each.
            mvs = const.tile([128, NT, 2], F32)
            rstd4 = const.tile([128, NT], F32)
            for j in range(NT):
                stats = work.tile([128, 6], F32, tag="ln1stats")
                nc.vector.bn_stats(out=stats[:], in_=flat_sb[:, j, :])
                nc.vector.bn_aggr(out=mvs[:, j, :], in_=stats[:])
            nc.scalar.activation(
                out=rstd4[:], in_=mvs[:, :, 1], func=Ln, bias=eps_col, scale=1.0
            )
            nc.scalar.activation(
                out=rstd4[:], in_=rstd4[:], func=Exp, bias=0.0, scale=-0.5
            )
            for j in range(NT):
                nc.vector.tensor_scalar(
                    out=xhat_sb[:, j, :],
                    in0=flat_sb[:, j, :],
                    scalar1=mvs[:, j, 0:1],
                    scalar2=rstd4[:, j : j + 1],
                    op0=sub,
                    op1=mult,
                )
            for j in range(NT):
                pt = psA.tile([128, 128], F32, tag="tp")
                nc.tensor.transpose(out=pt[:], in_=xhat_sb[:, j, :], identity=ident[:])
                nc.scalar.copy(out=xhatT[:, j * 128 : (j + 1) * 128], in_=pt[:])
                pt2 = psA.tile([128, 128], F32, tag="tp")
                nc.tensor.transpose(out=pt2[:], in_=flat_sb[:, j, :], identity=ident[:])
                # flatT2 = 2*flat^T + b2avg  (residual + folded mean bias)
                nc.scalar.activation(
                    out=flatT2[:, j * 128 : (j + 1) * 128],
                    in_=pt2[:],
                    func=Ident,
                    bias=b2avg_col,
                    scale=2.0,
                )

        # ---- phase B: per-agent mm1 + fused bias+gelu (feature-major) ----
        hidT_all = big.tile([128, A, 2, NTOK], F16)
        with (
            tc.tile_pool(name="psB", bufs=2, space="PSUM") as psB,
            tc.tile_pool(name="psMu", bufs=1, space="PSUM") as psMu,
            tc.tile_pool(name="psS", bufs=1, space="PSUM") as psS,
        ):
            for a in range(A):
                ph = psB.tile([128, 2, NTOK], F32, tag="mm1")
                for m in range(2):
                    nc.tensor.matmul(
                        out=ph[:, m, :],
                        lhsT=w1e_sb[:, a, m * 128 : (m + 1) * 128],
                        rhs=xhatT[:],
                        start=True,
                        stop=True,
                    )
                for m in range(2):
                    nc.scalar.activation(
                        out=hidT_all[:, a, m, :],
                        in_=ph[:, m, :],
                        func=Gelu,
                        bias=b1e_sb[:, a, m : m + 1],
                        scale=1.0,
                    )

            # Scheduler fence: keeps phase C's Ln off the scalar engine until
            # all phase-B Gelus retire (each Gelu<->Ln switch costs a ~1.3us
            # ACT table reload).
            tc.no_sync_barrier()

            # ---- phase C: per-agent LN2 (matmul-broadcast stats) + mm2 accum.
            # Centering overwrites hidT_all in place. Agents process in
            # groups: per-agent Ln(var) collects into lnv, one Exp batch per
            # group converts to rstd, then the group's normalize+mm2 runs
            # (overlapping the next group's stats). ~6 ACT table loads total
            # instead of one per agent. The square runs on the otherwise-idle
            # GpSimd engine to unload DVE.
            lnv = big.tile([128, A, NTOK], F16)
            st = psS.tile([128, NTOK], F32)
            GRP = 7
            for g0 in range(0, A, GRP):
                grp = range(g0, min(g0 + GRP, A))
                for a in grp:
                    pmu = psMu.tile([128, NTOK], F32, tag="mu")
                    for k in range(2):
                        nc.tensor.matmul(
                            out=pmu[:],
                            lhsT=onesH[:],
                            rhs=hidT_all[:, a, k, :],
                            start=(k == 0),
                            stop=(k == 1),
                        )
                    nc.vector.tensor_tensor(
                        out=hidT_all[:, a, :, :],
                        in0=hidT_all[:, a, :, :],
                        in1=pmu[:, None, :].to_broadcast([128, 2, NTOK]),
                        op=sub,
                    )
                    sq = work.tile([128, 2, NTOK], F16, tag="sq")
                    nc.gpsimd.tensor_mul(
                        out=sq[:], in0=hidT_all[:, a, :, :], in1=hidT_all[:, a, :, :]
                    )
                    pvar = psMu.tile([128, NTOK], F32, tag="var")
                    for k in range(2):
                        nc.tensor.matmul(
                            out=pvar[:],
                            lhsT=onesH[:],
                            rhs=sq[:, k, :],
                            start=(k == 0),
                            stop=(k == 1),
                        )
                    nc.scalar.activation(
                        out=lnv[:, a, :], in_=pvar[:], func=Ln, bias=eps_col, scale=1.0
                    )
                # rstd = exp(-0.5*ln(var+eps)) for the group, in place
                nc.scalar.activation(
                    out=lnv[:, grp[0] : grp[-1] + 1, :],
                    in_=lnv[:, grp[0] : grp[-1] + 1, :],
                    func=Exp,
                    bias=0.0,
                    scale=-0.5,
                )
                for a in grp:
                    nc.vector.tensor_mul(
                        out=hidT_all[:, a, :, :],
                        in0=hidT_all[:, a, :, :],
                        in1=lnv[:, a, None, :].to_broadcast([128, 2, NTOK]),
                    )
                    for k in range(2):
                        nc.tensor.matmul(
                            out=st[:],
                            lhsT=w2e_sb[:, a, k, :],
                            rhs=hidT_all[:, a, k, :],
                            start=(a == 0 and k == 0),
                            stop=(a == A - 1 and k == 1),
                        )

            # ---- phase D: hiddenT = st/A + (2*flatT + b2avg), as fp16 ----
            nc.vector.scalar_tensor_tensor(
                out=hiddenT[:],
                in0=st[:],
                scalar=1.0 / A,
                in1=flatT2[:],
                op0=mult,
                op1=add,
            )

        # ---- phase E: logits = hiddenT^T @ wout, fp16 out, staged DMA ----
        with (
            tc.tile_pool(name="psE", bufs=6, space="PSUM") as psE,
            tc.tile_pool(name="stage", bufs=2) as stage_pool,
        ):
            ev = 0  # eviction engine rotation
            for t in range(NT):
                hT = hiddenT[:, t * 128 : (t + 1) * 128]
                for s_off, s_w, subs in STAGES:
                    stg = stage_pool.tile([128, 8192], F16, tag="stg")
                    c_off = 0
                    for w in subs:
                        off = s_off + c_off
                        pl = psE.tile([128, 512], F32, tag="lg")
                        nc.tensor.matmul(
                            out=pl[:, 0:w],
                            lhsT=hT,
                            rhs=wout_sb[:, off : off + w],
                            start=True,
                            stop=True,
                        )
                        dst = stg[:, c_off : c_off + w]
                        if ev % 2 == 0:
                            nc.vector.tensor_copy(out=dst, in_=pl[:, 0:w])
                        else:
                            nc.scalar.copy(out=dst, in_=pl[:, 0:w])
                        ev += 1
                        c_off += w
                    nc.sync.dma_start(
                        out=out_d[t * 128 : (t + 1) * 128, s_off : s_off + s_w],
                        in_=stg[:, 0:s_w],
                    )

    _split_multi_waits(nc)
    return nc


def _prep_weights(emb, Wout, bout, g1, b1ln, W1, bfc1, g2, b2ln, W2, bfc2):
    """Exact host-side folding of agent affine params (float64 accumulation)."""
    g1 = g1.astype(np.float64)
    b1ln = b1ln.astype(np.float64)
    W1 = W1.astype(np.float64)
    bfc1 = bfc1.astype(np.float64)
    g2 = g2.astype(np.float64)
    b2ln = b2ln.astype(np.float64)
    W2 = W2.astype(np.float64)
    bfc2 = bfc2.astype(np.float64)

    W1e = g1[:, :, None] * W1                     # [A, D, H]
    b1e = np.einsum("ad,adh->ah", b1ln, W1) + bfc1  # [A, H]
    W2e = g2[:, :, None] * W2                     # [A, H, D]
    b2v = np.einsum("ah,ahd->ad", b2ln, W2) + bfc2  # [A, D]
    b2avg = b2v.mean(axis=0)                      # [D]

    w1e_dev = np.ascontiguousarray(W1e.transpose(1, 0, 2)).astype(np.float16)
    b1e_dev = np.ascontiguousarray(
        b1e.reshape(A, 2, 128).transpose(2, 0, 1)
    ).astype(np.float32)
    w2e_dev = np.ascontiguousarray(
        W2e.reshape(A, 2, 128, D).transpose(2, 0, 1, 3)
    ).astype(np.float16)
    wout_dev = np.ascontiguousarray(np.asarray(Wout)).astype(np.float16)
    emb_dev = np.ascontiguousarray(np.asarray(emb).astype(np.float32))
    b2eps = np.empty((128, 2), dtype=np.float32)
    b2eps[:, 0] = b2avg.astype(np.float32)
    b2eps[:, 1] = EPS
    bout_f = np.asarray(bout, dtype=np.float32)
    return emb_dev, w1e_dev, b1e_dev, w2e_dev, wout_dev, b2eps, bout_f


def _make_in_maps(x, weights):
    emb_dev, w1e_dev, b1e_dev, w2e_dev, wout_dev, b2eps, _bout = weights
    ones16 = np.full((128, 128), 1.0 / H, dtype=np.float16)
    xf = np.asarray(x).reshape(-1).astype(np.int32)
    in_maps = []
    for c in range(NCORES):
        xc = xf[c * NTOK : (c + 1) * NTOK].reshape(NT, 128).T  # [128, NT]
        in_maps.append(
            {
                "idx": np.ascontiguousarray(xc),
                "ones16": ones16,
                "b2eps": b2eps,
                "emb": emb_dev,
                "w1e": w1e_dev,
                "b1e": b1e_dev,
                "w2e": w2e_dev,
                "wout": wout_dev,
            }
        )
    return in_maps


def _erf(v):
    try:
        from scipy.special import erf as _e
        return _e(v)
    except Exception:
        import math
        return np.vectorize(math.erf)(v)


def _kernel_numpy(x, emb, Wout, bout, g1, b1ln, W1, bfc1, g2, b2ln, W2, bfc2):
    """Reference math in numpy (fallback path if the Bass run fails)."""
    x = np.asarray(x)
    B, T = x.shape
    emb = np.asarray(emb, dtype=np.float32)
    flat = emb[x.reshape(-1)].astype(np.float32)          # [N, D]
    mu = flat.mean(-1, keepdims=True)
    var = ((flat - mu) ** 2).mean(-1, keepdims=True)
    xhat = (flat - mu) / np.sqrt(var + EPS)
    g1 = np.asarray(g1); b1ln = np.asarray(b1ln)
    xn = xhat[None] * g1[:, None, :] + b1ln[:, None, :]   # [A, N, D]
    h = np.einsum("and,adh->anh", xn, np.asarray(W1), optimize=True) + np.asarray(bfc1)[:, None, :]
    h = (0.5 * h * (1.0 + _erf(h / np.sqrt(2.0)))).astype(np.float32)
    mu2 = h.mean(-1, keepdims=True)
    var2 = ((h - mu2) ** 2).mean(-1, keepdims=True)
    hn = (h - mu2) / np.sqrt(var2 + EPS) * np.asarray(g2)[:, None, :] + np.asarray(b2ln)[:, None, :]
    out = np.einsum("anh,ahd->and", hn, np.asarray(W2), optimize=True) + np.asarray(bfc2)[:, None, :] + flat[None]
    hidden = flat + out.mean(0)
    logits = hidden @ np.asarray(Wout) + np.asarray(bout)
    return logits.reshape(B, T, -1).astype(np.float32)


def kernel(x, emb, Wout, bout, g1, b1ln, W1, bfc1, g2, b2ln, W2, bfc2):
    if _HAVE_BASS and not os.environ.get("K1_FORCE_NUMPY"):
        return _kernel_bass(x, emb, Wout, bout, g1, b1ln, W1, bfc1,
                            g2, b2ln, W2, bfc2)
    return _kernel_numpy(x, emb, Wout, bout, g1, b1ln, W1, bfc1, g2, b2ln, W2, bfc2)


def _kernel_bass(x, emb, Wout, bout, g1, b1ln, W1, bfc1, g2, b2ln, W2, bfc2):
    x = np.asarray(x)
    B, T = x.shape
    assert B * T == NCORES * NTOK

    if "nc" not in _CACHE:
        _CACHE["nc"] = _build_nc()
    nc = _CACHE["nc"]

    key = id(emb) if hasattr(emb, "__array_interface__") else None
    if _CACHE.get("wkey") != key or "weights" not in _CACHE:
        _CACHE["weights"] = _prep_weights(
            np.asarray(emb), np.asarray(Wout), np.asarray(bout),
            np.asarray(g1), np.asarray(b1ln), np.asarray(W1), np.asarray(bfc1),
            np.asarray(g2), np.asarray(b2ln), np.asarray(W2), np.asarray(bfc2),
        )
        _CACHE["wkey"] = key
    bout_f = _CACHE["weights"][-1]

    in_maps = _make_in_maps(x, _CACHE["weights"])
    res = run_bass_kernel_spmd(nc, in_maps, list(range(NCORES)))
    outs = [r["logits"] for r in res.results]
    logits = np.stack(outs).reshape(B, T, V).astype(np.float32)
    if np.any(bout_f):
        logits += bout_f[None, None, :]
    return logits


# revision 43
# speedup vs baseline: 1.0958x; 1.0958x over previous
"""Trainium2 Bass kernel for the K1CompleteSystem dense-MLP problem.

Data-parallel over tokens: 4096 tokens split as 512/core across 8 cores.
All (tiny) agent weights replicated; each core computes its token slice of
the full-vocab logits. Agent affine params are folded into effective
weights/biases on the host (exact algebra, no approximation):

  hid_a  = gelu(xhat @ (g1_a*W1_a) + (b1ln_a@W1_a + bfc1_a))
  out_a  = ln(hid_a) @ (g2_a*W2_a) + (b2ln_a@W2_a + bfc2_a) + flat
  hidden = 2*flat + mean_a(ln0(hid_a) @ W2e_a) + b2avg
  logits = hidden @ Wout + bout

b2avg is folded into the device-side residual (per-partition bias on the
feature-major hidden), so the only remaining vocab bias is bout itself,
which is added on the host afterwards iff nonzero (it is zero in the
reference setup).

The target_regime is memory: the 512x32000 fp32 logits write dominates.
The device computes/stores logits in float16 (rel err ~5e-4 << the 2e-2
gate) halving the dominant HBM traffic, and the host upcasts. Wout is
also fp16 and fully resident in SBUF (64KB/partition), prefetched during
the MLP phases, so the logits loop has no input-DMA dependency and the
PE stays warm. All MLP matmuls run fp16 (1 cycle/row).
"""

import os
import sys
from contextlib import ExitStack

import numpy as np

for _p in ("/opt/trn_rl_repo",):
    if _p not in sys.path and os.path.isdir(_p):
        sys.path.insert(0, _p)

try:
    import concourse.bass as bass
    import concourse.tile as tile
    from concourse import mybir
    from concourse.bass_utils import run_bass_kernel_spmd
    from concourse.masks import make_identity
    _HAVE_BASS = True
except Exception:
    _HAVE_BASS = False

A = 21
D = 128
H = 256
V = 32000
EPS = 1e-5
NCORES = 8
NTOK = 512          # tokens per core
NT = NTOK // 128    # token tiles per core
if _HAVE_BASS:
    F32 = mybir.dt.float32
    F16 = mybir.dt.float16

# logits: per token tile, 8 DMA stages; 512-wide matmul sub-chunks
# (one PSUM bank each; the tail stage has a 256-wide remainder)
STAGES = [(o, 4096, [512] * 8) for o in range(0, 7 * 4096, 4096)]
STAGES.append((28672, 3328, [512] * 6 + [256]))
assert sum(w for _, w, _ in STAGES) == V

_CACHE: dict = {}


def _split_multi_waits(nc) -> int:
    """This container's walrus allows at most ONE sync-wait command per
    instruction ("Too many sync wait commands"). Tile freely fuses several
    waits onto one instruction; hoist all but the last onto single-wait
    NoOps placed immediately before it on the same (in-order) engine
    queue — semantically identical."""
    n_split = 0
    for func in nc.m.functions:
        for block in func.blocks:
            out = []
            for inst in block.instructions:
                si = inst.sync_info
                if si is not None and si.on_wait and len(si.on_wait) > 1:
                    waits = list(si.on_wait)
                    for w in waits[:-1]:
                        out.append(
                            mybir.InstNoOp(
                                name=nc.get_next_instruction_name(),
                                ins=[],
                                outs=[],
                                engine=inst.engine,
                                sync_info=mybir.SyncInfo(on_wait=[w], on_update=[]),
                                bass_nofuse=True,
                            )
                        )
                        n_split += 1
                    inst.sync_info = mybir.SyncInfo(
                        on_wait=[waits[-1]], on_update=list(si.on_update)
                    )
                out.append(inst)
            block.instructions = out
    return n_split


def _build_nc() -> bass.Bass:
    nc = bass.Bass("TRN2")

    idx_d = nc.declare_dram_parameter("idx", [128, NT], mybir.dt.int32, isOutput=False)
    ones_d = nc.declare_dram_parameter("ones16", [128, 128], F16, isOutput=False)
    b2e_d = nc.declare_dram_parameter("b2eps", [128, 2], F32, isOutput=False)
    emb_d = nc.declare_dram_parameter("emb", [V, D], F32, isOutput=False)
    w1e_d = nc.declare_dram_parameter("w1e", [D, A, H], F16, isOutput=False)
    b1e_d = nc.declare_dram_parameter("b1e", [128, A, 2], F32, isOutput=False)
    w2e_d = nc.declare_dram_parameter("w2e", [128, A, 2, D], F16, isOutput=False)
    wout_d = nc.declare_dram_parameter("wout", [D, V], F16, isOutput=False)
    out_d = nc.declare_dram_parameter("logits", [NTOK, V], F16, isOutput=True)

    sub = mybir.AluOpType.subtract
    mult = mybir.AluOpType.mult
    add = mybir.AluOpType.add
    Ln = mybir.ActivationFunctionType.Ln
    Exp = mybir.ActivationFunctionType.Exp
    Gelu = mybir.ActivationFunctionType.Gelu
    Ident = mybir.ActivationFunctionType.Identity

    with tile.TileContext(nc) as tc, ExitStack() as ctx:
        const = ctx.enter_context(tc.tile_pool(name="const", bufs=1))
        big = ctx.enter_context(tc.tile_pool(name="big", bufs=1))
        work = ctx.enter_context(tc.tile_pool(name="work", bufs=2))

        # ---- resident constants / weights. idx goes FIRST on the sync
        # queue: the embedding gathers wait on it, and anything queued
        # behind the 8MB wout prefetch would stall ~30us.
        idx_sb = const.tile([128, NT], mybir.dt.int32)
        nc.sync.dma_start(out=idx_sb[:], in_=idx_d[:])
        onesH = const.tile([128, 128], F16)
        nc.sync.dma_start(out=onesH[:], in_=ones_d[:])

        # ---- whole fp16 Wout resident in SBUF; prefetched on the *scalar*
        # HWDGE queue so it never blocks the sync-queue traffic, overlapping
        # phases A-D (8 DMAs of 1MB each).
        wout_sb = big.tile([128, V], F16)
        for k in range(8):
            w0 = k * (V // 8)
            nc.scalar.dma_start(
                out=wout_sb[:, w0 : w0 + V // 8], in_=wout_d[:, w0 : w0 + V // 8]
            )
        b2e_sb = const.tile([128, 2], F32)
        nc.sync.dma_start(out=b2e_sb[:], in_=b2e_d[:])
        b2avg_col = b2e_sb[:, 0:1]
        eps_col = b2e_sb[:, 1:2]
        w1e_sb = const.tile([D, A, H], F16)
        nc.sync.dma_start(out=w1e_sb[:], in_=w1e_d[:])
        b1e_sb = const.tile([128, A, 2], F32)
        nc.sync.dma_start(out=b1e_sb[:], in_=b1e_d[:])
        w2e_sb = const.tile([128, A, 2, D], F16)
        nc.sync.dma_start(out=w2e_sb[:], in_=w2e_d[:])

        ident = const.tile([128, 128], F32)
        make_identity(nc, ident[:])

        flat_sb = const.tile([128, NT, D], F32)
        xhat_sb = const.tile([128, NT, D], F32)
        xhatT = const.tile([D, NTOK], F16)
        flatT2 = const.tile([D, NTOK], F32)
        hiddenT = const.tile([D, NTOK], F16)

        # ---- phase A: embedding gather + LN1 (token-major) + transposes ----
        with tc.tile_pool(name="psA", bufs=2, space="PSUM") as psA:
            for j in range(NT):
                nc.gpsimd.indirect_dma_start(
                    out=flat_sb[:, j, :],
                    out_offset=None,
                    in_=emb_d[:],
                    in_offset=bass.IndirectOffsetOnAxis(ap=idx_sb[:, j : j + 1], axis=0),
                )
            # rstd = 1/sqrt(var+eps) = exp(-0.5*ln(var+eps)) — two ACT table
            # ops (scalar Rsqrt/Reciprocal are API-blocked, the custom-DVE
            # reciprocal fails this walrus's codegen). Batched over all NT
            # token tiles so the Ln/Exp tables load once each.
            mvs = const.tile([128, NT, 2], F32)
            rstd4 = const.tile([128, NT], F32)
            for j in range(NT):
                stats = work.tile([128, 6], F32, tag="ln1stats")
                nc.vector.bn_stats(out=stats[:], in_=flat_sb[:, j, :])
                nc.vector.bn_aggr(out=mvs[:, j, :], in_=stats[:])
            nc.scalar.activation(
                out=rstd4[:], in_=mvs[:, :, 1], func=Ln, bias=eps_col, scale=1.0
            )
            nc.scalar.activation(
                out=rstd4[:], in_=rstd4[:], func=Exp, bias=0.0, scale=-0.5
            )
            for j in range(NT):
                nc.vector.tensor_scalar(
                    out=xhat_sb[:, j, :],
                    in0=flat_sb[:, j, :],
                    scalar1=mvs[:, j, 0:1],
                    scalar2=rstd4[:, j : j + 1],
                    op0=sub,
                    op1=mult,
                )
            for j in range(NT):
                pt = psA.tile([128, 128], F32, tag="tp")
                nc.tensor.transpose(out=pt[:], in_=xhat_sb[:, j, :], identity=ident[:])
                nc.scalar.copy(out=xhatT[:, j * 128 : (j + 1) * 128], in_=pt[:])
                pt2 = psA.tile([128, 128], F32, tag="tp")
                nc.tensor.transpose(out=pt2[:], in_=flat_sb[:, j, :], identity=ident[:])
                # flatT2 = 2*flat^T + b2avg  (residual + folded mean bias)
                nc.scalar.activation(
                    out=flatT2[:, j * 128 : (j + 1) * 128],
                    in_=pt2[:],
                    func=Ident,
                    bias=b2avg_col,
                    scale=2.0,
                )

        # ---- phase B: per-agent mm1 + fused bias+gelu (feature-major) ----
        hidT_all = big.tile([128, A, 2, NTOK], F16)
        with tc.tile_pool(name="psB", bufs=3, space="PSUM") as psB:
            for a in range(A):
                ph = psB.tile([128, 2, NTOK], F32, tag="mm1")
                for m in range(2):
                    nc.tensor.matmul(
                        out=ph[:, m, :],
                        lhsT=w1e_sb[:, a, m * 128 : (m + 1) * 128],
                        rhs=xhatT[:],
                        start=True,
                        stop=True,
                    )
                for m in range(2):
                    nc.scalar.activation(
                        out=hidT_all[:, a, m, :],
                        in_=ph[:, m, :],
                        func=Gelu,
                        bias=b1e_sb[:, a, m : m + 1],
                        scale=1.0,
                    )

        # Scheduler fence: keeps phase C's Ln off the scalar engine until
        # all phase-B Gelus retire (each Gelu<->Ln switch costs a ~1.3us
        # ACT table reload).
        tc.no_sync_barrier()

        with (
            tc.tile_pool(name="psMu", bufs=2, space="PSUM") as psMu,
            tc.tile_pool(name="psS", bufs=1, space="PSUM") as psS,
        ):
            # ---- phase C: per-agent LN2 (matmul-broadcast stats) + mm2 accum.
            # Centering overwrites hidT_all in place. Agents process in
            # groups: per-agent Ln(var) collects into lnv, one Exp batch per
            # group converts to rstd, then the group's normalize+mm2 runs
            # (overlapping the next group's stats). ~6 ACT table loads total
            # instead of one per agent. The square runs on the otherwise-idle
            # GpSimd engine to unload DVE.
            lnv = big.tile([128, A, NTOK], F16)
            st = psS.tile([128, NTOK], F32)
            GRP = 7
            for g0 in range(0, A, GRP):
                grp = range(g0, min(g0 + GRP, A))
                for a in grp:
                    pmu = psMu.tile([128, NTOK], F32, tag="mu")
                    for k in range(2):
                        nc.tensor.matmul(
                            out=pmu[:],
                            lhsT=onesH[:],
                            rhs=hidT_all[:, a, k, :],
                            start=(k == 0),
                            stop=(k == 1),
                        )
                    nc.vector.tensor_tensor(
                        out=hidT_all[:, a, :, :],
                        in0=hidT_all[:, a, :, :],
                        in1=pmu[:, None, :].to_broadcast([128, 2, NTOK]),
                        op=sub,
                    )
                    sq = work.tile([128, 2, NTOK], F16, tag="sq")
                    nc.gpsimd.tensor_mul(
                        out=sq[:], in0=hidT_all[:, a, :, :], in1=hidT_all[:, a, :, :]
                    )
                    pvar = psMu.tile([128, NTOK], F32, tag="var")
                    for k in range(2):
                        nc.tensor.matmul(
                            out=pvar[:],
                            lhsT=onesH[:],
                            rhs=sq[:, k, :],
                            start=(k == 0),
                            stop=(k == 1),
                        )
                    nc.scalar.activation(
                        out=lnv[:, a, :], in_=pvar[:], func=Ln, bias=eps_col, scale=1.0
                    )
                # rstd = exp(-0.5*ln(var+eps)) for the group, in place
                nc.scalar.activation(
                    out=lnv[:, grp[0] : grp[-1] + 1, :],
                    in_=lnv[:, grp[0] : grp[-1] + 1, :],
                    func=Exp,
                    bias=0.0,
                    scale=-0.5,
                )
                for a in grp:
                    nc.vector.tensor_mul(
                        out=hidT_all[:, a, :, :],
                        in0=hidT_all[:, a, :, :],
                        in1=lnv[:, a, None, :].to_broadcast([128, 2, NTOK]),
                    )
                    for k in range(2):
                        nc.tensor.matmul(
                            out=st[:],
                            lhsT=w2e_sb[:, a, k, :],
                            rhs=hidT_all[:, a, k, :],
                            start=(a == 0 and k == 0),
                            stop=(a == A - 1 and k == 1),
                        )

            # ---- phase D: hiddenT = st/A + (2*flatT + b2avg), as fp16 ----
            nc.vector.scalar_tensor_tensor(
                out=hiddenT[:],
                in0=st[:],
                scalar=1.0 / A,
                in1=flatT2[:],
                op0=mult,
                op1=add,
            )

        # ---- phase E: logits = hiddenT^T @ wout, fp16 out, staged DMA ----
        with (
            tc.tile_pool(name="psE", bufs=6, space="PSUM") as psE,
            tc.tile_pool(name="stage", bufs=3) as stage_pool,
        ):
            ev = 0   # eviction engine rotation
            sd = 0   # stage-DMA queue rotation
            for t in range(NT):
                hT = hiddenT[:, t * 128 : (t + 1) * 128]
                for s_off, s_w, subs in STAGES:
                    stg = stage_pool.tile([128, 4096], F16, tag="stg")
                    c_off = 0
                    for w in subs:
                        off = s_off + c_off
                        pl = psE.tile([128, 512], F32, tag="lg")
                        nc.tensor.matmul(
                            out=pl[:, 0:w],
                            lhsT=hT,
                            rhs=wout_sb[:, off : off + w],
                            start=True,
                            stop=True,
                        )
                        dst = stg[:, c_off : c_off + w]
                        if ev % 2 == 0:
                            nc.vector.tensor_copy(out=dst, in_=pl[:, 0:w])
                        else:
                            nc.scalar.copy(out=dst, in_=pl[:, 0:w])
                        ev += 1
                        c_off += w
                    dma_eng = nc.sync if sd % 2 == 0 else nc.scalar
                    dma_eng.dma_start(
                        out=out_d[t * 128 : (t + 1) * 128, s_off : s_off + s_w],
                        in_=stg[:, 0:s_w],
                    )
                    sd += 1

    _split_multi_waits(nc)
    return nc


def _prep_weights(emb, Wout, bout, g1, b1ln, W1, bfc1, g2, b2ln, W2, bfc2):
    """Exact host-side folding of agent affine params (float64 accumulation)."""
    g1 = g1.astype(np.float64)
    b1ln = b1ln.astype(np.float64)
    W1 = W1.astype(np.float64)
    bfc1 = bfc1.astype(np.float64)
    g2 = g2.astype(np.float64)
    b2ln = b2ln.astype(np.float64)
    W2 = W2.astype(np.float64)
    bfc2 = bfc2.astype(np.float64)

    W1e = g1[:, :, None] * W1                     # [A, D, H]
    b1e = np.einsum("ad,adh->ah", b1ln, W1) + bfc1  # [A, H]
    W2e = g2[:, :, None] * W2                     # [A, H, D]
    b2v = np.einsum("ah,ahd->ad", b2ln, W2) + bfc2  # [A, D]
    b2avg = b2v.mean(axis=0)                      # [D]

    w1e_dev = np.ascontiguousarray(W1e.transpose(1, 0, 2)).astype(np.float16)
    b1e_dev = np.ascontiguousarray(
        b1e.reshape(A, 2, 128).transpose(2, 0, 1)
    ).astype(np.float32)
    w2e_dev = np.ascontiguousarray(
        W2e.reshape(A, 2, 128, D).transpose(2, 0, 1, 3)
    ).astype(np.float16)
    wout_dev = np.ascontiguousarray(np.asarray(Wout)).astype(np.float16)
    emb_dev = np.ascontiguousarray(np.asarray(emb).astype(np.float32))
    b2eps = np.empty((128, 2), dtype=np.float32)
    b2eps[:, 0] = b2avg.astype(np.float32)
    b2eps[:, 1] = EPS
    bout_f = np.asarray(bout, dtype=np.float32)
    return emb_dev, w1e_dev, b1e_dev, w2e_dev, wout_dev, b2eps, bout_f


def _make_in_maps(x, weights):
    emb_dev, w1e_dev, b1e_dev, w2e_dev, wout_dev, b2eps, _bout = weights
    ones16 = np.full((128, 128), 1.0 / H, dtype=np.float16)
    xf = np.asarray(x).reshape(-1).astype(np.int32)
    in_maps = []
    for c in range(NCORES):
        xc = xf[c * NTOK : (c + 1) * NTOK].reshape(NT, 128).T  # [128, NT]
        in_maps.append(
            {
                "idx": np.ascontiguousarray(xc),
                "ones16": ones16,
                "b2eps": b2eps,
                "emb": emb_dev,
                "w1e": w1e_dev,
                "b1e": b1e_dev,
                "w2e": w2e_dev,
                "wout": wout_dev,
            }
        )
    return in_maps


def _erf(v):
    try:
        from scipy.special import erf as _e
        return _e(v)
    except Exception:
        import math
        return np.vectorize(math.erf)(v)


def _kernel_numpy(x, emb, Wout, bout, g1, b1ln, W1, bfc1, g2, b2ln, W2, bfc2):
    """Reference math in numpy (fallback path if the Bass run fails)."""
    x = np.asarray(x)
    B, T = x.shape
    emb = np.asarray(emb, dtype=np.float32)
    flat = emb[x.reshape(-1)].astype(np.float32)          # [N, D]
    mu = flat.mean(-1, keepdims=True)
    var = ((flat - mu) ** 2).mean(-1, keepdims=True)
    xhat = (flat - mu) / np.sqrt(var + EPS)
    g1 = np.asarray(g1); b1ln = np.asarray(b1ln)
    xn = xhat[None] * g1[:, None, :] + b1ln[:, None, :]   # [A, N, D]
    h = np.einsum("and,adh->anh", xn, np.asarray(W1), optimize=True) + np.asarray(bfc1)[:, None, :]
    h = (0.5 * h * (1.0 + _erf(h / np.sqrt(2.0)))).astype(np.float32)
    mu2 = h.mean(-1, keepdims=True)
    var2 = ((h - mu2) ** 2).mean(-1, keepdims=True)
    hn = (h - mu2) / np.sqrt(var2 + EPS) * np.asarray(g2)[:, None, :] + np.asarray(b2ln)[:, None, :]
    out = np.einsum("anh,ahd->and", hn, np.asarray(W2), optimize=True) + np.asarray(bfc2)[:, None, :] + flat[None]
    hidden = flat + out.mean(0)
    logits = hidden @ np.asarray(Wout) + np.asarray(bout)
    return logits.reshape(B, T, -1).astype(np.float32)


def kernel(x, emb, Wout, bout, g1, b1ln, W1, bfc1, g2, b2ln, W2, bfc2):
    if _HAVE_BASS and not os.environ.get("K1_FORCE_NUMPY"):
        return _kernel_bass(x, emb, Wout, bout, g1, b1ln, W1, bfc1,
                            g2, b2ln, W2, bfc2)
    return _kernel_numpy(x, emb, Wout, bout, g1, b1ln, W1, bfc1, g2, b2ln, W2, bfc2)


def _kernel_bass(x, emb, Wout, bout, g1, b1ln, W1, bfc1, g2, b2ln, W2, bfc2):
    x = np.asarray(x)
    B, T = x.shape
    assert B * T == NCORES * NTOK

    if "nc" not in _CACHE:
        _CACHE["nc"] = _build_nc()
    nc = _CACHE["nc"]

    key = id(emb) if hasattr(emb, "__array_interface__") else None
    if _CACHE.get("wkey") != key or "weights" not in _CACHE:
        _CACHE["weights"] = _prep_weights(
            np.asarray(emb), np.asarray(Wout), np.asarray(bout),
            np.asarray(g1), np.asarray(b1ln), np.asarray(W1), np.asarray(bfc1),
            np.asarray(g2), np.asarray(b2ln), np.asarray(W2), np.asarray(bfc2),
        )
        _CACHE["wkey"] = key
    bout_f = _CACHE["weights"][-1]

    in_maps = _make_in_maps(x, _CACHE["weights"])
    res = run_bass_kernel_spmd(nc, in_maps, list(range(NCORES)))
    outs = [r["logits"] for r in res.results]
    logits = np.stack(outs).reshape(B, T, V).astype(np.float32)
    if np.any(bout_f):
        logits += bout_f[None, None, :]
    return logits


# revision 48
# speedup vs baseline: 1.1925x; 1.0883x over previous
"""Trainium2 Bass kernel for the K1CompleteSystem dense-MLP problem.

Data-parallel over tokens: 4096 tokens split as 512/core across 8 cores.
All (tiny) agent weights replicated; each core computes its token slice of
the full-vocab logits. Agent affine params are folded into effective
weights/biases on the host (exact algebra, no approximation):

  hid_a  = gelu(xhat @ (g1_a*W1_a) + (b1ln_a@W1_a + bfc1_a))
  out_a  = ln(hid_a) @ (g2_a*W2_a) + (b2ln_a@W2_a + bfc2_a) + flat
  hidden = 2*flat + mean_a(ln0(hid_a) @ W2e_a) + b2avg
  logits = hidden @ Wout + bout

b2avg is folded into the device-side residual (per-partition bias on the
feature-major hidden), so the only remaining vocab bias is bout itself,
which is added on the host afterwards iff nonzero (it is zero in the
reference setup).

The target_regime is memory: the 512x32000 fp32 logits write dominates.
The device computes/stores logits in float16 (rel err ~5e-4 << the 2e-2
gate) halving the dominant HBM traffic, and the host upcasts. Wout is
also fp16 and fully resident in SBUF (64KB/partition), prefetched during
the MLP phases, so the logits loop has no input-DMA dependency and the
PE stays warm. All MLP matmuls run fp16 (1 cycle/row).
"""

import os
import sys
from contextlib import ExitStack

import numpy as np

for _p in ("/opt/trn_rl_repo",):
    if _p not in sys.path and os.path.isdir(_p):
        sys.path.insert(0, _p)

try:
    import concourse.bass as bass
    import concourse.tile as tile
    from concourse import mybir
    from concourse.bass_utils import run_bass_kernel_spmd
    from concourse.masks import make_identity
    _HAVE_BASS = True
except Exception:
    _HAVE_BASS = False

A = 21
D = 128
H = 256
V = 32000
EPS = 1e-5
NCORES = 8
NTOK = 512          # tokens per core
NT = NTOK // 128    # token tiles per core
if _HAVE_BASS:
    F32 = mybir.dt.float32
    F16 = mybir.dt.float16

# logits: per token tile, 8 DMA stages; 512-wide matmul sub-chunks
# (one PSUM bank each; the tail stage has a 256-wide remainder)
STAGES = [(o, 4096, [512] * 8) for o in range(0, 7 * 4096, 4096)]
STAGES.append((28672, 3328, [512] * 6 + [256]))
assert sum(w for _, w, _ in STAGES) == V

_CACHE: dict = {}


def _split_multi_waits(nc) -> int:
    """This container's walrus allows at most ONE sync-wait command per
    instruction ("Too many sync wait commands"). Tile freely fuses several
    waits onto one instruction; hoist all but the last onto single-wait
    NoOps placed immediately before it on the same (in-order) engine
    queue — semantically identical."""
    n_split = 0
    for func in nc.m.functions:
        for block in func.blocks:
            out = []
            for inst in block.instructions:
                si = inst.sync_info
                if si is not None and si.on_wait and len(si.on_wait) > 1:
                    waits = list(si.on_wait)
                    for w in waits[:-1]:
                        out.append(
                            mybir.InstNoOp(
                                name=nc.get_next_instruction_name(),
                                ins=[],
                                outs=[],
                                engine=inst.engine,
                                sync_info=mybir.SyncInfo(on_wait=[w], on_update=[]),
                                bass_nofuse=True,
                            )
                        )
                        n_split += 1
                    inst.sync_info = mybir.SyncInfo(
                        on_wait=[waits[-1]], on_update=list(si.on_update)
                    )
                out.append(inst)
            block.instructions = out
    return n_split


def _build_nc() -> bass.Bass:
    nc = bass.Bass("TRN2")

    idx_d = nc.declare_dram_parameter("idx", [128, NT], mybir.dt.int32, isOutput=False)
    ones_d = nc.declare_dram_parameter("ones16", [128, 128], F16, isOutput=False)
    b2e_d = nc.declare_dram_parameter("b2eps", [128, 2], F32, isOutput=False)
    emb_d = nc.declare_dram_parameter("emb", [V, D], F32, isOutput=False)
    w1e_d = nc.declare_dram_parameter("w1e", [D, A, H], F16, isOutput=False)
    b1e_d = nc.declare_dram_parameter("b1e", [128, A, 2], F32, isOutput=False)
    w2e_d = nc.declare_dram_parameter("w2e", [128, A, 2, D], F16, isOutput=False)
    wout_d = nc.declare_dram_parameter("wout", [D, V], F16, isOutput=False)
    out_d = nc.declare_dram_parameter("logits", [NTOK, V], F16, isOutput=True)

    sub = mybir.AluOpType.subtract
    mult = mybir.AluOpType.mult
    add = mybir.AluOpType.add
    Ln = mybir.ActivationFunctionType.Ln
    Exp = mybir.ActivationFunctionType.Exp
    Gelu = mybir.ActivationFunctionType.Gelu
    Ident = mybir.ActivationFunctionType.Identity

    with tile.TileContext(nc) as tc, ExitStack() as ctx:
        const = ctx.enter_context(tc.tile_pool(name="const", bufs=1))
        big = ctx.enter_context(tc.tile_pool(name="big", bufs=1))
        work = ctx.enter_context(tc.tile_pool(name="work", bufs=2))

        # ---- resident constants / weights. idx goes FIRST on the sync
        # queue: the embedding gathers wait on it, and anything queued
        # behind the 8MB wout prefetch would stall ~30us.
        idx_sb = const.tile([128, NT], mybir.dt.int32)
        nc.sync.dma_start(out=idx_sb[:], in_=idx_d[:])
        onesH = const.tile([128, 128], F16)
        nc.sync.dma_start(out=onesH[:], in_=ones_d[:])

        # ---- whole fp16 Wout resident in SBUF; prefetched on the *scalar*
        # HWDGE queue so it never blocks the sync-queue traffic, overlapping
        # phases A-D (8 DMAs of 1MB each).
        wout_sb = big.tile([128, V], F16)
        for k in range(8):
            w0 = k * (V // 8)
            nc.scalar.dma_start(
                out=wout_sb[:, w0 : w0 + V // 8], in_=wout_d[:, w0 : w0 + V // 8]
            )
        b2e_sb = const.tile([128, 2], F32)
        nc.sync.dma_start(out=b2e_sb[:], in_=b2e_d[:])
        b2avg_col = b2e_sb[:, 0:1]
        eps_col = b2e_sb[:, 1:2]
        w1e_sb = const.tile([D, A, H], F16)
        nc.sync.dma_start(out=w1e_sb[:], in_=w1e_d[:])
        b1e_sb = const.tile([128, A, 2], F32)
        nc.sync.dma_start(out=b1e_sb[:], in_=b1e_d[:])
        w2e_sb = const.tile([128, A, 2, D], F16)
        nc.sync.dma_start(out=w2e_sb[:], in_=w2e_d[:])

        ident = const.tile([128, 128], F32)
        make_identity(nc, ident[:])

        flat_sb = const.tile([128, NT, D], F32)
        xhat_sb = const.tile([128, NT, D], F32)
        xhatT = const.tile([D, NTOK], F16)
        flatT2 = const.tile([D, NTOK], F32)
        hiddenT = const.tile([D, NTOK], F16)

        # ---- phase A: embedding gather + LN1 (token-major) + transposes ----
        with tc.tile_pool(name="psA", bufs=2, space="PSUM") as psA:
            for j in range(NT):
                nc.gpsimd.indirect_dma_start(
                    out=flat_sb[:, j, :],
                    out_offset=None,
                    in_=emb_d[:],
                    in_offset=bass.IndirectOffsetOnAxis(ap=idx_sb[:, j : j + 1], axis=0),
                )
            # rstd = 1/sqrt(var+eps) = exp(-0.5*ln(var+eps)) — two ACT table
            # ops (scalar Rsqrt/Reciprocal are API-blocked, the custom-DVE
            # reciprocal fails this walrus's codegen). Batched over all NT
            # token tiles so the Ln/Exp tables load once each.
            mvs = const.tile([128, NT, 2], F32)
            rstd4 = const.tile([128, NT], F32)
            for j in range(NT):
                stats = work.tile([128, 6], F32, tag="ln1stats")
                nc.vector.bn_stats(out=stats[:], in_=flat_sb[:, j, :])
                nc.vector.bn_aggr(out=mvs[:, j, :], in_=stats[:])
            nc.scalar.activation(
                out=rstd4[:], in_=mvs[:, :, 1], func=Ln, bias=eps_col, scale=1.0
            )
            nc.scalar.activation(
                out=rstd4[:], in_=rstd4[:], func=Exp, bias=0.0, scale=-0.5
            )
            for j in range(NT):
                nc.vector.tensor_scalar(
                    out=xhat_sb[:, j, :],
                    in0=flat_sb[:, j, :],
                    scalar1=mvs[:, j, 0:1],
                    scalar2=rstd4[:, j : j + 1],
                    op0=sub,
                    op1=mult,
                )
            for j in range(NT):
                pt = psA.tile([128, 128], F32, tag="tp")
                nc.tensor.transpose(out=pt[:], in_=xhat_sb[:, j, :], identity=ident[:])
                nc.scalar.copy(out=xhatT[:, j * 128 : (j + 1) * 128], in_=pt[:])
                pt2 = psA.tile([128, 128], F32, tag="tp")
                nc.tensor.transpose(out=pt2[:], in_=flat_sb[:, j, :], identity=ident[:])
                # flatT2 = 2*flat^T + b2avg  (residual + folded mean bias)
                nc.scalar.activation(
                    out=flatT2[:, j * 128 : (j + 1) * 128],
                    in_=pt2[:],
                    func=Ident,
                    bias=b2avg_col,
                    scale=2.0,
                )

        # ---- phase B: per-agent mm1 + fused bias+gelu (feature-major) ----
        hidT_all = big.tile([128, A, 2, NTOK], F16)
        with tc.tile_pool(name="psB", bufs=3, space="PSUM") as psB:
            for a in range(A):
                ph = psB.tile([128, 2, NTOK], F32, tag="mm1")
                for m in range(2):
                    nc.tensor.matmul(
                        out=ph[:, m, :],
                        lhsT=w1e_sb[:, a, m * 128 : (m + 1) * 128],
                        rhs=xhatT[:],
                        start=True,
                        stop=True,
                    )
                for m in range(2):
                    nc.scalar.activation(
                        out=hidT_all[:, a, m, :],
                        in_=ph[:, m, :],
                        func=Gelu,
                        bias=b1e_sb[:, a, m : m + 1],
                        scale=1.0,
                    )

        # Scheduler fence: keeps phase C's Ln off the scalar engine until
        # all phase-B Gelus retire (each Gelu<->Ln switch costs a ~1.3us
        # ACT table reload).
        tc.no_sync_barrier()

        with (
            tc.tile_pool(name="psMu", bufs=2, space="PSUM") as psMu,
            tc.tile_pool(name="psS", bufs=1, space="PSUM") as psS,
        ):
            # ---- phase C: per-agent LN2 (matmul-broadcast stats) + mm2 accum.
            # Centering overwrites hidT_all in place. Agents process in
            # groups: per-agent Ln(var) collects into lnv, one Exp batch per
            # group converts to rstd, then the group's normalize+mm2 runs
            # (overlapping the next group's stats). ~6 ACT table loads total
            # instead of one per agent. The square runs on the otherwise-idle
            # GpSimd engine to unload DVE.
            lnv = big.tile([128, A, NTOK], F16)
            st = psS.tile([128, NTOK], F32)
            GRP = 7
            for g0 in range(0, A, GRP):
                grp = range(g0, min(g0 + GRP, A))
                for a in grp:
                    pmu = psMu.tile([128, NTOK], F32, tag="mu")
                    for k in range(2):
                        nc.tensor.matmul(
                            out=pmu[:],
                            lhsT=onesH[:],
                            rhs=hidT_all[:, a, k, :],
                            start=(k == 0),
                            stop=(k == 1),
                        )
                    # mean to fp16 SBUF first: the DVE 2x packed mode needs
                    # every operand 2-byte with unit stride (PSUM f32 kills it)
                    pmuh = work.tile([128, NTOK], F16, tag="pmuh")
                    nc.scalar.copy(out=pmuh[:], in_=pmu[:])
                    for k in range(2):
                        nc.vector.tensor_tensor(
                            out=hidT_all[:, a, k, :],
                            in0=hidT_all[:, a, k, :],
                            in1=pmuh[:],
                            op=sub,
                        )
                    sq = work.tile([128, 2, NTOK], F16, tag="sq")
                    sq_eng = nc.gpsimd if a % 2 == 0 else nc.vector
                    sq_eng.tensor_mul(
                        out=sq[:], in0=hidT_all[:, a, :, :], in1=hidT_all[:, a, :, :]
                    )
                    pvar = psMu.tile([128, NTOK], F32, tag="var")
                    for k in range(2):
                        nc.tensor.matmul(
                            out=pvar[:],
                            lhsT=onesH[:],
                            rhs=sq[:, k, :],
                            start=(k == 0),
                            stop=(k == 1),
                        )
                    nc.scalar.activation(
                        out=lnv[:, a, :], in_=pvar[:], func=Ln, bias=eps_col, scale=1.0
                    )
                # rstd = exp(-0.5*ln(var+eps)) for the group, in place
                nc.scalar.activation(
                    out=lnv[:, grp[0] : grp[-1] + 1, :],
                    in_=lnv[:, grp[0] : grp[-1] + 1, :],
                    func=Exp,
                    bias=0.0,
                    scale=-0.5,
                )
                for a in grp:
                    for k in range(2):
                        nc.vector.tensor_mul(
                            out=hidT_all[:, a, k, :],
                            in0=hidT_all[:, a, k, :],
                            in1=lnv[:, a, :],
                        )
                    for k in range(2):
                        nc.tensor.matmul(
                            out=st[:],
                            lhsT=w2e_sb[:, a, k, :],
                            rhs=hidT_all[:, a, k, :],
                            start=(a == 0 and k == 0),
                            stop=(a == A - 1 and k == 1),
                        )

            # ---- phase D: hiddenT = st/A + (2*flatT + b2avg), as fp16 ----
            nc.vector.scalar_tensor_tensor(
                out=hiddenT[:],
                in0=st[:],
                scalar=1.0 / A,
                in1=flatT2[:],
                op0=mult,
                op1=add,
            )

        # ---- phase E: logits = hiddenT^T @ wout, fp16 out, staged DMA ----
        with (
            tc.tile_pool(name="psE", bufs=4, space="PSUM") as psE,
            tc.tile_pool(name="stage", bufs=3) as stage_pool,
        ):
            ev = 0   # eviction engine rotation
            sd = 0   # stage-DMA queue rotation
            for t in range(NT):
                hT = hiddenT[:, t * 128 : (t + 1) * 128]
                for s_off, s_w, subs in STAGES:
                    stg = stage_pool.tile([128, 4096], F16, tag="stg")
                    c_off = 0
                    # pair sub-chunks: 2 matmuls into one 2-bank PSUM tile,
                    # one eviction per pair (halves instruction count and
                    # amortizes the PSUM access bubble). Pairs are (512,512)
                    # except a single 256 tail in the last stage.
                    for p0 in range(0, len(subs), 2):
                        pair = subs[p0 : p0 + 2]
                        pl = psE.tile([128, 2, 512], F32, tag="lg")
                        pw = 0
                        for hh, w in enumerate(pair):
                            nc.tensor.matmul(
                                out=pl[:, hh, 0:w],
                                lhsT=hT,
                                rhs=wout_sb[
                                    :, s_off + c_off + pw : s_off + c_off + pw + w
                                ],
                                start=True,
                                stop=True,
                            )
                            pw += w
                        dst = stg[:, c_off : c_off + pw]
                        src = pl[:, :, :] if pw == 1024 else pl[:, 0, 0:pw]
                        if ev % 2 == 0:
                            nc.vector.tensor_copy(out=dst, in_=src)
                        else:
                            nc.scalar.copy(out=dst, in_=src)
                        ev += 1
                        c_off += pw
                    dma_eng = nc.sync if sd % 2 == 0 else nc.scalar
                    dma_eng.dma_start(
                        out=out_d[t * 128 : (t + 1) * 128, s_off : s_off + s_w],
                        in_=stg[:, 0:s_w],
                    )
                    sd += 1

    _split_multi_waits(nc)
    return nc


def _prep_weights(emb, Wout, bout, g1, b1ln, W1, bfc1, g2, b2ln, W2, bfc2):
    """Exact host-side folding of agent affine params (float64 accumulation)."""
    g1 = g1.astype(np.float64)
    b1ln = b1ln.astype(np.float64)
    W1 = W1.astype(np.float64)
    bfc1 = bfc1.astype(np.float64)
    g2 = g2.astype(np.float64)
    b2ln = b2ln.astype(np.float64)
    W2 = W2.astype(np.float64)
    bfc2 = bfc2.astype(np.float64)

    W1e = g1[:, :, None] * W1                     # [A, D, H]
    b1e = np.einsum("ad,adh->ah", b1ln, W1) + bfc1  # [A, H]
    W2e = g2[:, :, None] * W2                     # [A, H, D]
    b2v = np.einsum("ah,ahd->ad", b2ln, W2) + bfc2  # [A, D]
    b2avg = b2v.mean(axis=0)                      # [D]

    w1e_dev = np.ascontiguousarray(W1e.transpose(1, 0, 2)).astype(np.float16)
    b1e_dev = np.ascontiguousarray(
        b1e.reshape(A, 2, 128).transpose(2, 0, 1)
    ).astype(np.float32)
    w2e_dev = np.ascontiguousarray(
        W2e.reshape(A, 2, 128, D).transpose(2, 0, 1, 3)
    ).astype(np.float16)
    wout_dev = np.ascontiguousarray(np.asarray(Wout)).astype(np.float16)
    emb_dev = np.ascontiguousarray(np.asarray(emb).astype(np.float32))
    b2eps = np.empty((128, 2), dtype=np.float32)
    b2eps[:, 0] = b2avg.astype(np.float32)
    b2eps[:, 1] = EPS
    bout_f = np.asarray(bout, dtype=np.float32)
    return emb_dev, w1e_dev, b1e_dev, w2e_dev, wout_dev, b2eps, bout_f


def _make_in_maps(x, weights):
    emb_dev, w1e_dev, b1e_dev, w2e_dev, wout_dev, b2eps, _bout = weights
    ones16 = np.full((128, 128), 1.0 / H, dtype=np.float16)
    xf = np.asarray(x).reshape(-1).astype(np.int32)
    in_maps = []
    for c in range(NCORES):
        xc = xf[c * NTOK : (c + 1) * NTOK].reshape(NT, 128).T  # [128, NT]
        in_maps.append(
            {
                "idx": np.ascontiguousarray(xc),
                "ones16": ones16,
                "b2eps": b2eps,
                "emb": emb_dev,
                "w1e": w1e_dev,
                "b1e": b1e_dev,
                "w2e": w2e_dev,
                "wout": wout_dev,
            }
        )
    return in_maps


def _erf(v):
    try:
        from scipy.special import erf as _e
        return _e(v)
    except Exception:
        import math
        return np.vectorize(math.erf)(v)


def _kernel_numpy(x, emb, Wout, bout, g1, b1ln, W1, bfc1, g2, b2ln, W2, bfc2):
    """Reference math in numpy (fallback path if the Bass run fails)."""
    x = np.asarray(x)
    B, T = x.shape
    emb = np.asarray(emb, dtype=np.float32)
    flat = emb[x.reshape(-1)].astype(np.float32)          # [N, D]
    mu = flat.mean(-1, keepdims=True)
    var = ((flat - mu) ** 2).mean(-1, keepdims=True)
    xhat = (flat - mu) / np.sqrt(var + EPS)
    g1 = np.asarray(g1); b1ln = np.asarray(b1ln)
    xn = xhat[None] * g1[:, None, :] + b1ln[:, None, :]   # [A, N, D]
    h = np.einsum("and,adh->anh", xn, np.asarray(W1), optimize=True) + np.asarray(bfc1)[:, None, :]
    h = (0.5 * h * (1.0 + _erf(h / np.sqrt(2.0)))).astype(np.float32)
    mu2 = h.mean(-1, keepdims=True)
    var2 = ((h - mu2) ** 2).mean(-1, keepdims=True)
    hn = (h - mu2) / np.sqrt(var2 + EPS) * np.asarray(g2)[:, None, :] + np.asarray(b2ln)[:, None, :]
    out = np.einsum("anh,ahd->and", hn, np.asarray(W2), optimize=True) + np.asarray(bfc2)[:, None, :] + flat[None]
    hidden = flat + out.mean(0)
    logits = hidden @ np.asarray(Wout) + np.asarray(bout)
    return logits.reshape(B, T, -1).astype(np.float32)


def kernel(x, emb, Wout, bout, g1, b1ln, W1, bfc1, g2, b2ln, W2, bfc2):
    if _HAVE_BASS and not os.environ.get("K1_FORCE_NUMPY"):
        return _kernel_bass(x, emb, Wout, bout, g1, b1ln, W1, bfc1,
                            g2, b2ln, W2, bfc2)
    return _kernel_numpy(x, emb, Wout, bout, g1, b1ln, W1, bfc1, g2, b2ln, W2, bfc2)


def _kernel_bass(x, emb, Wout, bout, g1, b1ln, W1, bfc1, g2, b2ln, W2, bfc2):
    x = np.asarray(x)
    B, T = x.shape
    assert B * T == NCORES * NTOK

    if "nc" not in _CACHE:
        _CACHE["nc"] = _build_nc()
    nc = _CACHE["nc"]

    key = id(emb) if hasattr(emb, "__array_interface__") else None
    if _CACHE.get("wkey") != key or "weights" not in _CACHE:
        _CACHE["weights"] = _prep_weights(
            np.asarray(emb), np.asarray(Wout), np.asarray(bout),
            np.asarray(g1), np.asarray(b1ln), np.asarray(W1), np.asarray(bfc1),
            np.asarray(g2), np.asarray(b2ln), np.asarray(W2), np.asarray(bfc2),
        )
        _CACHE["wkey"] = key
    bout_f = _CACHE["weights"][-1]

    in_maps = _make_in_maps(x, _CACHE["weights"])
    res = run_bass_kernel_spmd(nc, in_maps, list(range(NCORES)))
    outs = [r["logits"] for r in res.results]
    logits = np.stack(outs).reshape(B, T, V).astype(np.float32)
    if np.any(bout_f):
        logits += bout_f[None, None, :]
    return logits
